# revision 25
# baseline (speedup 1.0000x reference)
"""Trainium2 Bass kernel for nn_DecoderHead (B=2, T=2048, D=1024, H=16, DH=64).

y = x + softmax_causal((x @ Wq.T) split to heads @ k^T / sqrt(D)) @ v

Sharding: 8 cores = 2 (batch) x 4 (head groups of 4 heads). Each core computes
its batch's q-projection for its 256 output features (Wq column-sharded by
head), causal attention for its 4 heads, adds the residual slice, and writes a
[T, 256] slice; the host concatenates slices (the all-gather over the
head-split d dim is a free host-side assembly).

Per-core dataflow (all matmul contractions on the PE partition axis):
  qT[e, t]   = sum_d WqT[d, e] * xT[d, t]         (q projection, transposed)
  sT[tk, tq] = sum_dh kT_h[dh, tk] * qT_h[dh, tq] (scores, transposed; two
               heads run concurrently in distinct PE row-groups since DH=64)
  eT         = exp(sT / 32) * tri on diag chunks  (ACT exp, DVE mask only on
                                                   the partial 128x128 chunks)
  oT[dh', tq]= sum_tk vO[tk, dh'] * eT[tk, tq]    (vO = [v | ones]; row 64
                                                   accumulates the denominator)
  y[tq, dh]  = transpose(oT) / denom + x_res      (PE transpose into its own
                                                   PSUM ring, fused DVE epi)

Schedule is tq-tile-major with per-tile interleave: load stage c+1's inputs,
project q for tile c, then run both head-pairs' attention for tile c — the PE
never sits behind more DMA than one stage. Causality is exploited at 128-key
granularity on the diagonal: QK / exp / PV all restrict their moving range to
the visible queries, and only the partial diagonal chunk gets a mask-multiply.
"""

import os
from collections import deque

import numpy as np

import concourse.bass as bass
import concourse.mybir as mybir
import concourse.tile as tile
from concourse import bacc
from concourse.alu_op_type import AluOpType
from concourse.bass_utils import run_bass_kernel_spmd

# Problem shape (hardcoded per the harness contract).
B, T, D, H = 2, 2048, 1024, 16
DH = D // H          # 64
N_CORES = 8
HPC = H // (N_CORES // B)   # heads per core = 4
EPC = HPC * DH       # output features per core = 256
P = 128              # SBUF partitions
TQ = 512             # query-tile width (matmul moving-dim)
NTQ = T // TQ        # 4
NTKB = T // P        # 16 key blocks of 128
DT = D // P          # 8 contraction tiles for the q projection
EG = EPC // P        # 2 head-pair groups of 128 e-rows
SCALE = 1.0 / np.sqrt(np.float32(D))   # 1/32 (reference scales by sqrt(d))

F32 = mybir.dt.float32

# Matmul operand dtype: bf16 (full PE rate, ~1.5e-4 rel err), fp32r (fp32 w/
# 11-bit mantissa), fp32 (exact, 1/4 rate). "pv8" = bf16 q/k + fp8e4m3
# DoubleRow PV (2 key blocks per matmul at 2x fp8 ALU rate).
VARIANT = os.environ.get("DH_VARIANT", "pv8")


def _mm_dt(variant):
    return {
        "fp32": mybir.dt.float32,
        "fp32r": mybir.dt.float32r,
        "fp32r_bx": mybir.dt.float32r,   # bf16 q-projection inputs
        "bf16": mybir.dt.bfloat16,
        "pv8": mybir.dt.bfloat16,
    }[variant]


def _is_pv8(variant):
    return variant == "pv8"


def _x_dt(variant):
    return mybir.dt.bfloat16 if variant == "fp32r_bx" else _mm_dt(variant)


def _np_round_fp32r(a: np.ndarray) -> np.ndarray:
    """Round fp32 to the fp32r value set: 11-bit mantissa, RNE, low 12 bits 0."""
    u = a.astype(np.float32).view(np.uint32)
    lsb = (u >> np.uint32(12)) & np.uint32(1)
    r = (u + np.uint32(0x7FF) + lsb) & np.uint32(0xFFFFF000)
    return r.view(np.float32)


def _host_cast(a: np.ndarray, variant: str) -> np.ndarray:
    a = np.ascontiguousarray(a, dtype=np.float32)
    if variant == "fp32r":
        return _np_round_fp32r(a)
    if variant in ("bf16", "pv8"):
        import ml_dtypes
        return a.astype(ml_dtypes.bfloat16)
    return a


def _host_cast_fp8(a: np.ndarray) -> np.ndarray:
    return np.ascontiguousarray(a, dtype=np.float32).astype(
        mybir.dt.np(mybir.dt.float8e4)
    )


def _chunk_pair_ap(et, base):
    """AP over the two partial diagonal chunks of an et tile [P, 2, TQ]:
    elements at flat free offsets base+[0,128) and base+640+[0,128)
    (u=0 chunk m0 and u=1 chunk m0+1, stride 640)."""
    flat = et.rearrange("p u t -> p (u t)")
    ck = flat[:, base:2 * TQ:640].unsqueeze(-1)
    ck.ap[2] = [1, P]
    return ck


def build_nc(variant: str = VARIANT, repeat: int = 1):
    """Build the per-core SPMD Bass program. `repeat` wraps the body in a
    hardware loop (timing only)."""
    mdt = _mm_dt(variant)
    xdt = _x_dt(variant)
    pv8 = _is_pv8(variant)
    edt = mybir.dt.float8e4 if pv8 else mdt   # et / vO / mask dtype
    nc = bacc.Bacc(
        "TRN2", target_bir_lowering=False, debug=False, num_devices=N_CORES
    )

    xT = nc.dram_tensor("xT", [D, T], xdt, kind="ExternalInput").ap()
    wqT = nc.dram_tensor("wqT", [D, EPC], xdt, kind="ExternalInput").ap()
    kT = nc.dram_tensor("kT", [P, EG, T], mdt, kind="ExternalInput").ap()
    if pv8:
        vO = nc.dram_tensor(
            "vO", [P, NTKB // 2, HPC, 2, 80], edt, kind="ExternalInput"
        ).ap()
        tri = nc.dram_tensor("tri", [P, 2, 2 * P], edt, kind="ExternalInput").ap()
    else:
        vO = nc.dram_tensor(
            "vO", [P, NTKB, HPC, DH + 1], mdt, kind="ExternalInput"
        ).ap()
        tri = nc.dram_tensor("tri", [P, P], mdt, kind="ExternalInput").ap()
    xres = nc.dram_tensor("xres", [P, T // P, EPC], xdt, kind="ExternalInput").ap()
    ident = nc.dram_tensor("ident", [P, P], F32, kind="ExternalInput").ap()
    y = nc.dram_tensor("y", [T, EPC], F32, kind="ExternalOutput").ap()

    with tile.TileContext(nc) as tc:
        with (
            tc.tile_pool(name="const", bufs=1) as cpool,
            tc.tile_pool(name="xq", bufs=1) as xqpool,
            tc.tile_pool(name="work", bufs=6) as wpool,
            tc.tile_pool(name="epi", bufs=2) as epool,
            tc.tile_pool(name="ps_s", bufs=3, space="PSUM") as ps_s,
            tc.tile_pool(name="ps_o", bufs=2, space="PSUM") as ps_o,
        ):
            def body(_iv=None):
                # ---- tiles -------------------------------------------------
                id_sb = cpool.tile([P, P], F32, name="id_sb", tag="id_sb", bufs=2)
                tri_sb = cpool.tile(
                    [P, 2, 2 * P] if pv8 else [P, P], edt,
                    name="tri_sb", tag="tri_sb", bufs=2,
                )
                wq_sb = xqpool.tile([P, DT, EPC], xdt, name="wq_sb", tag="wq_sb", bufs=2)
                xT_sb = xqpool.tile([P, DT, T], xdt, name="xT_sb", tag="xT_sb")
                kT_sb = cpool.tile([P, EG, T], mdt, name="kT_sb", tag="kT_sb", bufs=2)
                vO_sb = cpool.tile(
                    [P, NTKB // 2, HPC, 2, 80] if pv8
                    else [P, NTKB, HPC, DH + 1],
                    edt, name="vO_sb", tag="vO_sb", bufs=2,
                )
                xr_sb = cpool.tile([P, T // P, EPC], xdt, name="xr_sb",
                                   tag="xr_sb")
                qT_sb = xqpool.tile([P, EG, T], mdt, name="qT_sb", tag="qT_sb")

                # ---- stage-0 loads ----------------------------------------
                nc.sync.dma_start(id_sb[:], ident[:])
                nc.sync.dma_start(tri_sb[:], tri[:])

                def load_xq(c):
                    """q-projection inputs for tq-tile c."""
                    sl = bass.ts(c, TQ)
                    for dt_i in range(DT):
                        if c == 0:
                            nc.sync.dma_start(
                                wq_sb[:, dt_i, :], wqT[dt_i * P:(dt_i + 1) * P, :]
                            )
                        nc.sync.dma_start(
                            xT_sb[:, dt_i, sl], xT[dt_i * P:(dt_i + 1) * P, sl]
                        )

                def load_att(c):
                    """Attention inputs first needed by tq-tile c."""
                    sl = bass.ts(c, TQ)
                    nc.sync.dma_start(kT_sb[:, :, sl], kT[:, :, sl])
                    if pv8:
                        nc.sync.dma_start(
                            vO_sb[:, 2 * c:2 * (c + 1)], vO[:, 2 * c:2 * (c + 1)]
                        )
                    else:
                        nc.sync.dma_start(
                            vO_sb[:, 4 * c:4 * (c + 1)], vO[:, 4 * c:4 * (c + 1)]
                        )
                    nc.sync.dma_start(
                        xr_sb[:, 4 * c:4 * (c + 1)], xres[:, 4 * c:4 * (c + 1)]
                    )

                load_xq(0)
                load_att(0)

                # Warm-up while stage-0 DMA streams: prime the ACT exp table
                # and keep PE busy so the HAM clock-gate opens (dummy work on
                # the identity tile; results unused).
                warm_et = wpool.tile([P, P], F32, name="warm_et", tag="warm", bufs=1)
                psw = ps_o.tile([P, P], F32, name="psw", tag="o")
                warm_in = tri_sb[:, 0, 0:P] if pv8 else tri_sb[:]
                for w in range(16):
                    nc.tensor.matmul(
                        psw[:], warm_in, warm_in, start=True, stop=True,
                    )
                nc.scalar.activation(
                    warm_et[:], psw[:],
                    mybir.ActivationFunctionType.Exp, scale=0.01,
                )

                pending = deque()

                def epilogue_start(h, tqt, pso_t):
                    oT = epool.tile([DH + 1, TQ], F32, name="oT", tag="oT",
                                    bufs=3)
                    nc.vector.tensor_copy(oT[:], pso_t[:])
                    return (h, tqt, oT)

                def epilogue(state):
                    h, tqt, oT = state
                    ysb = epool.tile([P, 4, DH], F32, name="ysb", tag="ysb")
                    pst = ps_s.tile([P, 4, DH + 1], F32, name="pst", tag="s")
                    for j in range(4):
                        nc.tensor.transpose(
                            pst[:, j, :],
                            oT[:, j * P:(j + 1) * P],
                            id_sb[0:DH + 1, 0:DH + 1],
                        )
                    rc = epool.tile([P, 4], F32, name="rc", tag="rc", bufs=4)
                    nc.vector.reciprocal(rc[:], pst[:, :, DH])
                    rcb = rc[:, :].unsqueeze(-1).broadcast_to([P, 4, DH])
                    nc.vector.tensor_mul(ysb[:], pst[:, :, 0:DH], rcb)
                    nc.vector.tensor_add(
                        ysb[:], ysb[:],
                        xr_sb[:, 4 * tqt:4 * (tqt + 1), h * DH:(h + 1) * DH],
                    )
                    ydst = y[tqt * TQ:(tqt + 1) * TQ, h * DH:(h + 1) * DH]
                    # y stores ride the gpsimd SW-DGE queue so they never
                    # compete with the sync queue's input loads (and the idle
                    # gpsimd FIFO can't form a cross-engine wait cycle).
                    nc.gpsimd.dma_start(
                        ydst.rearrange("(j p) c -> p j c", p=P), ysb[:]
                    )

                def attention(hp, tqt):
                    g = hp
                    ntk = 4 * (tqt + 1)
                    npairs = ntk // 2
                    tq0 = tqt * TQ
                    pso2 = [
                        ps_o.tile([DH + 1, TQ], F32, name=f"pso{i}", tag="o")
                        for i in range(2)
                    ]

                    def emit_pv(p_et2, p_pair, last=False):
                        if pv8:
                            lo = max(0, P * (2 * p_pair - 4 * tqt))
                            for i in range(2):
                                nc.tensor.matmul(
                                    pso2[i][:, lo:TQ],
                                    vO_sb[:, p_pair, 2 * hp + i, :, 0:DH + 1],
                                    p_et2[i][:, :, lo:TQ],
                                    start=(p_pair == 0),
                                    stop=last,
                                    skip_group_check=True,
                                    perf_mode=mybir.MatmulPerfMode.DoubleRow,
                                )
                            return
                        for u in range(2):
                            tkb = 2 * p_pair + u
                            lo = max(0, P * (tkb - 4 * tqt))
                            for i in range(2):
                                nc.tensor.matmul(
                                    pso2[i][:, lo:TQ],
                                    vO_sb[:, tkb, 2 * hp + i, :],
                                    p_et2[i][:, u, lo:TQ],
                                    start=(tkb == 0),
                                    stop=(last and u == 1),
                                    skip_group_check=True,
                                )

                    prev = None
                    for pair in range(npairs):
                        # Visible-query lower bound for this pair's first
                        # block (the second block keeps the same bound so one
                        # rectangular exp covers both).
                        lo = max(0, P * (2 * pair - 4 * tqt))
                        et2 = []
                        pssc2 = [
                            ps_s.tile([P, 2, TQ], F32, name=f"pssc{i}", tag="s")
                            for i in range(2)
                        ]
                        for i in range(2):
                            bp = DH * i
                            for u in range(2):
                                tkb = 2 * pair + u
                                nc.tensor.matmul(
                                    pssc2[i][:, u, lo:TQ],
                                    kT_sb[bp:bp + DH, g,
                                          tkb * P:(tkb + 1) * P],
                                    qT_sb[bp:bp + DH, g, tq0 + lo:tq0 + TQ],
                                    start=True,
                                    stop=True,
                                )
                        for i in range(2):
                            if pv8:
                                et = wpool.tile([P, 2, TQ], edt,
                                                name=f"et{i}", tag="et", bufs=6)
                                nc.scalar.activation(
                                    et[:, :, lo:TQ], pssc2[i][:, :, lo:TQ],
                                    mybir.ActivationFunctionType.Exp,
                                    scale=float(SCALE),
                                )
                                if 2 * pair >= 4 * tqt:   # diagonal pair
                                    m0 = 2 * pair - 4 * tqt
                                    ck = et[:, :, P * m0:P * m0 + 2 * P]
                                    nc.vector.tensor_mul(ck, ck, tri_sb[:])
                            else:
                                et = wpool.tile([P, 2, TQ], mdt,
                                                name=f"et{i}", tag="et", bufs=6)
                                nc.scalar.activation(
                                    et[:, :, lo:TQ], pssc2[i][:, :, lo:TQ],
                                    mybir.ActivationFunctionType.Exp,
                                    scale=float(SCALE),
                                )
                                if 2 * pair >= 4 * tqt:   # diagonal pair
                                    m0 = 2 * pair - 4 * tqt
                                    ck = _chunk_pair_ap(et, P * m0)
                                    trib = tri_sb[:, :].unsqueeze(1).broadcast_to(
                                        [P, 2, P]
                                    )
                                    nc.vector.tensor_mul(ck, ck, trib)
                            et2.append(et)
                        if prev is not None:
                            emit_pv(*prev)
                        prev = (et2, pair)
                        if pending:
                            epilogue(pending.popleft())
                    emit_pv(*prev, last=True)
                    for i in range(2):
                        pending.append(epilogue_start(2 * hp + i, tqt, pso2[i]))

                def qproj(c, g):
                    sl = bass.ts(c, TQ)
                    psq = ps_s.tile([P, TQ], F32, name="psq", tag="s")
                    for dt_i in range(DT):
                        nc.tensor.matmul(
                            psq[:],
                            wq_sb[:, dt_i, g * P:(g + 1) * P],
                            xT_sb[:, dt_i, sl],
                            start=(dt_i == 0),
                            stop=(dt_i == DT - 1),
                        )
                    nc.vector.tensor_copy(qT_sb[:, g, sl], psq[:])

                # ---- main schedule: per tile, stage c+1 loads + both
                # head-pairs' attention, with the NEXT tile's q-projection for
                # head-pair g emitted right after attention g (so it overlaps
                # the other pair's exp chain and never stalls the QK queue at
                # a tile boundary). qproj shares the "s" PSUM ring.
                # All four tiles' q-projections run up front: in the repeat
                # loop their inputs are prefetched during the previous
                # iteration, so this is ~7us of dense PE work bridging the
                # iteration boundary (keeps the HAM clock-gate open).
                for tqc in range(NTQ):
                    if tqc > 0:
                        load_xq(tqc)
                    qproj(tqc, 0)
                    qproj(tqc, 1)
                for tqc in range(1, NTQ):
                    load_att(tqc)
                for tqc in range(NTQ):
                    attention(0, tqc)
                    attention(1, tqc)
                while pending:
                    epilogue(pending.popleft())

            if repeat == 1:
                body()
            else:
                tc.For_i_unrolled(0, repeat, 1, body, max_unroll=1)

    nc.compile()
    return nc


def prep_in_maps(x, k, v, Wq, variant: str = VARIANT):
    """Build the 8 per-core input maps from full inputs (host-side numpy)."""
    x = np.asarray(x, dtype=np.float32)
    k = np.asarray(k, dtype=np.float32)
    v = np.asarray(v, dtype=np.float32)
    Wq = np.asarray(Wq, dtype=np.float32)

    # tri[i, j] = 1 where key i is visible to query j within a diagonal chunk
    tri = (np.arange(P)[:, None] <= np.arange(P)[None, :]).astype(np.float32)
    pv8 = _is_pv8(variant)
    if pv8:
        # Diagonal-pair mask over [2 u-blocks x 2 chunks x P]: for block-pair
        # (m0, m0+1) over query chunks (m0, m0+1): u=0 -> [tri | ones],
        # u=1 -> [zeros | tri].
        tri4 = np.empty((P, 2, 2, P), dtype=np.float32)
        tri4[:, 0, 0] = tri
        tri4[:, 0, 1] = 1.0
        tri4[:, 1, 0] = 0.0
        tri4[:, 1, 1] = tri
        tri_in = _host_cast_fp8(tri4.reshape(P, 2, 2 * P))
    ident = np.eye(P, dtype=np.float32)

    in_maps = []
    for c in range(N_CORES):
        b = c // (N_CORES // B)
        grp = c % (N_CORES // B)
        heads = slice(HPC * grp, HPC * (grp + 1))
        cols = slice(EPC * grp, EPC * (grp + 1))

        xT_c = x[b].T                                   # [D, T]
        wqT_c = Wq[cols, :].T                           # [D, EPC]
        kT_c = np.zeros((P, EG, T), dtype=np.float32)
        for lh in range(HPC):
            kT_c[DH * (lh % 2):DH * (lh % 2) + DH, lh // 2, :] = \
                k[b, HPC * grp + lh].T
        vv = v[b, heads]                                # [HPC, T, DH]
        vO_c = np.ones((P, NTKB, HPC, DH + 1), dtype=np.float32)
        vO_c[:, :, :, :DH] = vv.reshape(HPC, NTKB, P, DH).transpose(2, 1, 0, 3)
        if pv8:
            # block-pair layout [P, NTKB//2, HPC, 2, 80]: the two 128-key
            # blocks of a pair side by side, free dim padded 65 -> 80 so the
            # DoubleRow weight AP's pair-stride is 16-byte aligned.
            vp = np.zeros((P, NTKB // 2, HPC, 2, 80), dtype=np.float32)
            vp[:, :, :, :, :DH + 1] = vO_c.reshape(
                P, NTKB // 2, 2, HPC, DH + 1
            ).transpose(0, 1, 3, 2, 4)
            vO_c = vp
        xres_c = np.ascontiguousarray(
            x[b][:, cols].reshape(NTKB, P, EPC).transpose(1, 0, 2)
        )
        xvar = "bf16" if variant == "fp32r_bx" else variant
        in_maps.append({
            "xT": _host_cast(xT_c, xvar),
            "wqT": _host_cast(wqT_c, xvar),
            "kT": _host_cast(kT_c, variant),
            "vO": _host_cast_fp8(vO_c) if pv8 else _host_cast(vO_c, variant),
            "xres": _host_cast(xres_c, xvar),
            "tri": tri_in if pv8 else _host_cast(tri, variant),
            "ident": ident,
        })
    return in_maps


def gather_output(results):
    """Assemble full [B, T, D] output from 8 per-core [T, EPC] slices."""
    y = np.empty((B, T, D), dtype=np.float32)
    for c in range(N_CORES):
        b = c // (N_CORES // B)
        grp = c % (N_CORES // B)
        y[b, :, EPC * grp:EPC * (grp + 1)] = results[c]["y"]
    return y


_NC_CACHE = {}


def kernel(x, k, v, Wq):
    key = (VARIANT, 1)
    if key not in _NC_CACHE:
        _NC_CACHE[key] = build_nc(VARIANT, repeat=1)
    nc = _NC_CACHE[key]
    in_maps = prep_in_maps(x, k, v, Wq, VARIANT)
    res = run_bass_kernel_spmd(nc, in_maps, core_ids=list(range(N_CORES)))
    return gather_output(res.results)
